# revision 27
# baseline (speedup 1.0000x reference)
"""Trainium2 Bass kernel for nn_ModelIAS_53618371724066 (segment_reduce).

Computes, for each batch row b:
    logits = hidden[b, 1:, :] @ W + b_vec          # [T, S]
    merged[w, :] = mean over {t : seg[b,t] == w} of logits[t, :]   (0 if empty)
    out[b] = merged.T                               # [S, T]

Strategy (data-parallel over batch, 32 rows per core on 8 cores):
  - hidden is quantized host-side to fp8 e3m4 with a per-token scale s_t
    (absmax -> 14.0) so input DMA traffic halves vs fp16; W stays fp16
    (its small magnitudes fall into the e3m4 denormal range).  The matmul
    runs mixed fp8 x fp16 at the bf16 stream rate with fp8 FWL weight
    loads.  Measured end-to-end max rel err ~1.4e-2 vs the 2e-2 gate.
  - The scatter matrix Mg[t, w] = (seg[t] == w) * g[t]/s_t is built in
    ONE DVE tensor_scalar per t-chunk using the two-op form
    (is_equal then mult) -- folding the mean weight g and the fp8
    de-scale into Mg.  This makes the PSUM->SBUF logits copy a pure
    cast, done as a single ACT instruction per row over [128, 2, S].
  - Stage 1 (PE): logits[t_chunk, s] = sum_k hidT[k-chunk].T @ W[k-chunk]
    accumulated in one fp32 PSUM bank per row ([128, 2, S]).
  - Stage 2 (PE): out[s, w] = sum_c lsb_c.T @ Mg_c.  s-channels 0:128 go
    to po1 [128, T]; the 130-wide remainder (2 channels) accumulates into
    a shared [2, 2, T] bank per row-pair so its PSUM->SBUF copy and DMA
    amortize over 2 rows.
  - Outputs leave the chip in fp16 (host converts to f32): po1 is cast by
    DVE into a [128, 4, T] staging tile (one DMA per 4 rows), po2 by ACT.
  - A ~4us burst of dummy matmuls at program start warms the PE HAM clock
    gate (1.2 -> 2.4 GHz) before real work arrives.
  - All hidden DMAs are issued up front on the GPSIMD ring (whole input
    fits SBUF); outputs go on the SP ring; constants on the ACT ring.
  - Per-instruction sem-waits are legalized for the pinned walrus by
    _split_sync_waits.
"""

import numpy as np
import ml_dtypes

import concourse.bass as bass
import concourse.tile as tile
from concourse import mybir
from concourse.bass_utils import run_bass_kernel_spmd

B, T, H, S = 256, 256, 768, 130
N_CORES = 8
RPC = B // N_CORES  # rows per core
KCH = H // 128  # k chunks of the hidden dim
F32 = mybir.dt.float32
HP = mybir.dt.float16
FP8 = mybir.dt.float8e3  # e3m4
N_WARM = 8
ROWS_PER_HDMA = 4
HDMA_GROUPS = [2, 2, 4, 4, 4, 4, 4, 4, 4]


def _split_sync_waits(nc):
    """The pinned walrus build rejects instructions carrying more than one
    sync-wait command ("Too many sync wait commands", setupSyncWait).  Keep
    one wait per instruction and hoist the rest onto NoOps inserted just
    before it on the same engine (same semantics: all waits still execute
    before the instruction, in stream order)."""
    for f in nc.m.functions:
        for blk in f.blocks:
            il = blk.instructions
            i = 0
            while i < len(il):
                inst = il[i]
                si = inst.sync_info
                if si is not None and si.on_wait and len(si.on_wait) >= 2:
                    waits = list(si.on_wait)
                    keep = [waits.pop()]
                    pos = i
                    for j, w in enumerate(waits):
                        nop = mybir.InstNoOp(name=f"{inst.name}_ws{j}", ins=[], outs=[])
                        nop.engine = inst.engine
                        nop.sync_info = mybir.SyncInfo(on_wait=[w], on_update=[])
                        il.insert(pos, nop)
                        pos += 1
                        i += 1
                    inst.sync_info = mybir.SyncInfo(
                        on_wait=keep, on_update=list(si.on_update)
                    )
                i += 1


def _build_program(rpc=RPC, with_bias=False, split_waits=True):
    nc = bass.Bass("TRN2", target_bir_lowering=False, debug=False)

    hid = nc.dram_tensor("hiddent", [128, rpc, KCH, T], FP8, kind="ExternalInput")
    w_d = nc.dram_tensor("w", [128, KCH, S], HP, kind="ExternalInput")
    b_d = nc.dram_tensor("bvec", [1, S], HP, kind="ExternalInput")
    seg_d = nc.dram_tensor("segt", [128, 2, rpc], F32, kind="ExternalInput")
    g_d = nc.dram_tensor("gst", [128, 2, rpc], F32, kind="ExternalInput")
    out1_d = nc.dram_tensor("out1", [128, rpc, T], HP, kind="ExternalOutput")
    out2_d = nc.dram_tensor("out2", [2, rpc, T], HP, kind="ExternalOutput")

    eq = mybir.AluOpType.is_equal
    mul = mybir.AluOpType.mult
    copyf = mybir.ActivationFunctionType.Copy
    assert rpc % ROWS_PER_HDMA == 0
    ngrp = rpc // ROWS_PER_HDMA
    with tile.TileContext(nc) as tc:
        with (
            tc.tile_pool(name="const", bufs=1) as const_pool,
            tc.tile_pool(name="hid", bufs=len(HDMA_GROUPS)) as hid_pool,
            tc.tile_pool(name="mgp", bufs=3) as mg_pool,
            tc.tile_pool(name="lsbp", bufs=3) as lsb_pool,
            tc.tile_pool(name="ob1p", bufs=3) as ob1_pool,
            tc.tile_pool(name="ob2p", bufs=4) as ob2_pool,
            tc.tile_pool(name="warm", bufs=1, space=bass.MemorySpace.PSUM) as warm_pool,
            tc.tile_pool(name="psl", bufs=2, space=bass.MemorySpace.PSUM) as psl_pool,
            tc.tile_pool(name="po1", bufs=2, space=bass.MemorySpace.PSUM) as po1_pool,
            tc.tile_pool(name="po2", bufs=2, space=bass.MemorySpace.PSUM) as po2_pool,
        ):
            # --- HAM warmup: keep the PE busy from t~0 so the clock gate
            # opens (K=8/8) before real matmuls arrive.  Dummy data.
            wz = const_pool.tile([128, 512], HP)
            nc.vector.memset(wz[:], 0.0)
            pwu = warm_pool.tile([64, 512], F32)
            for i in range(N_WARM):
                nc.tensor.matmul(
                    pwu[:, :],
                    wz[:, 0:64],
                    wz[:, :],
                    start=True,
                    stop=True,
                    skip_group_check=True,
                )

            # --- all input DMAs ride ONE SP-ring queue in priority order:
            # tiny constants first (so per-row deps clear by ~9us), then the
            # whole hidden tensor.  The first two hidden chunks are 2 rows
            # each so row 0 can start as early as possible.  The SP engine
            # has no per-row duties, so trigger instructions blocking on
            # DMA-ring capacity cost nothing.
            hts = []  # (row_start, n_rows, tile)
            ht0 = hid_pool.tile([128, HDMA_GROUPS[0], KCH, T], FP8, tag="ht0", name="htt")
            nc.sync.dma_start(ht0[:], hid.ap()[:, 0 : HDMA_GROUPS[0]])
            hts.append((0, HDMA_GROUPS[0], ht0))
            wt = const_pool.tile([128, KCH, S], HP)
            nc.sync.dma_start(wt[:], w_d.ap()[:])
            # tiny per-row constants ride the otherwise-idle GPSIMD ring so
            # they don't queue behind hidden chunks
            segt = const_pool.tile([128, 2, rpc], F32)
            nc.gpsimd.dma_start(segt[:], seg_d.ap()[:])
            gst = const_pool.tile([128, 2, rpc], F32)
            nc.gpsimd.dma_start(gst[:], g_d.ap()[:])
            row0 = HDMA_GROUPS[0]
            for nrow in HDMA_GROUPS[1:]:
                ht = hid_pool.tile([128, nrow, KCH, T], FP8, tag=f"ht{nrow}", name="htt")
                nc.sync.dma_start(ht[:], hid.ap()[:, row0 : row0 + nrow])
                hts.append((row0, nrow, ht))
                row0 += nrow
            assert row0 == rpc

            def hid_slice(r):
                for row0, nrow, ht in hts:
                    if row0 <= r < row0 + nrow:
                        return ht[:, r - row0]
                raise AssertionError

            # dummy ACTIVATE pulls the lazy ACT table load into the idle
            # startup window instead of the first row's critical path
            wz2 = const_pool.tile([1, 8], HP)
            nc.scalar.copy(wz2[:], wz[0:1, 0:8])
            iota_i = const_pool.tile([128, T], mybir.dt.int32)
            nc.gpsimd.iota(iota_i[:], pattern=[[1, T]], base=0, channel_multiplier=0)
            iota16 = const_pool.tile([128, T], HP)
            nc.vector.tensor_copy(iota16[:], iota_i[:])
            if with_bias:
                ones = const_pool.tile([1, 128], HP)
                nc.vector.memset(ones[:], 1.0)
                bsb = const_pool.tile([1, S], HP)
                nc.scalar.dma_start(bsb[:], b_d.ap()[:])

            pending = []
            po2s = {}
            ob1s = {}
            ob2s = {}

            def emit_stage2(item):
                pr, plsb, pmg = item
                pair, rr2 = divmod(pr, 2)
                g4, rr4 = divmod(pr, 4)
                po1 = po1_pool.tile([128, T], F32, tag="po1")
                if rr2 == 0:
                    po2s[pair] = po2_pool.tile([128, 2, T], F32, tag="po2", name="po2t")
                po2 = po2s[pair]
                # po2 uses a full-width stationary (s-channels 2:130) so all
                # stage-2 matmuls are unmasked M=128 and pipeline back-to-back;
                # only its last 2 partitions (s=128,129) are kept.
                for c in range(2):
                    nc.tensor.matmul(
                        po1[:],
                        plsb[:, c, 0:128],
                        pmg[:, c, :],
                        start=(c == 0),
                        stop=(c == 1),
                        skip_group_check=True,
                    )
                for c in range(2):
                    nc.tensor.matmul(
                        po2[:, rr2, :],
                        plsb[:, c, 2:S],
                        pmg[:, c, :],
                        start=(c == 0),
                        stop=(c == 1),
                        skip_group_check=True,
                    )
                g8, rr8 = divmod(pr, 8)
                if rr8 == 0:
                    ob1s[g8] = ob1_pool.tile([128, 8, T], HP, tag="ob1", name="ob1t")
                ob1 = ob1s[g8]
                nc.vector.tensor_copy(ob1[:, rr8, :], po1[:])
                if g8 == 3 and rr8 == 3:
                    # split the last group's DMAs so the final transfer is small
                    nc.sync.dma_start(
                        out1_d.ap()[:, 24:28], ob1[:, 0:4, :]
                    )
                if g8 == 3 and rr8 == 5:
                    nc.sync.dma_start(
                        out1_d.ap()[:, 28:30], ob1[:, 4:6, :]
                    )
                if rr8 == 7:
                    if g8 == 3:
                        nc.sync.dma_start(
                            out1_d.ap()[:, 30:32], ob1[:, 6:8, :]
                        )
                    else:
                        nc.sync.dma_start(
                            out1_d.ap()[:, 8 * g8 : 8 * (g8 + 1)], ob1[:]
                        )
                    del ob1s[g8]
                if rr2 == 1:
                    if rr4 == 1:
                        ob2s[g4] = ob2_pool.tile([128, 4, T], HP, tag="ob2", name="ob2t")
                    ob2 = ob2s[g4]
                    nc.scalar.copy(ob2[:, 2 * (pair % 2) : 2 * (pair % 2) + 2, :], po2[:])
                    del po2s[pair]
                    if rr4 == 3:
                        nc.sync.dma_start(
                            out2_d.ap()[:, 4 * g4 : 4 * (g4 + 1)],
                            ob2[126:128, :, :],
                        )
                        del ob2s[g4]

            for r in range(rpc):
                ht_r = hid_slice(r)

                # Mg[t, w] = (seg[t] == w) * g[t]/s_t in fp16, one DVE
                # two-op tensor_scalar per t-chunk
                mg = mg_pool.tile([128, 2, T], HP, tag="mg")
                for c in range(2):
                    nc.vector.tensor_scalar(
                        mg[:, c, :],
                        iota16[:],
                        segt[:, c, r : r + 1],
                        gst[:, c, r : r + 1],
                        eq,
                        mul,
                    )

                # stage 1: scaled logits for both t-chunks into one PSUM bank.
                # stage 2 of the previous row is emitted between the two
                # t-chunks so its lsb/Mg dependencies have settled by then.
                psl = psl_pool.tile([128, 2, S], F32, tag="psl")
                for c in range(2):
                    for k in range(KCH):
                        nc.tensor.matmul(
                            psl[:, c, :],
                            ht_r[:, k, 128 * c : 128 * (c + 1)],
                            wt[:, k, :],
                            start=(k == 0),
                            stop=(k == KCH - 1 and not with_bias),
                            skip_group_check=True,
                        )
                    if with_bias:
                        nc.tensor.matmul(
                            psl[:, c, :], ones[:], bsb[:], start=False, stop=True,
                            skip_group_check=True,
                        )
                    if c == 0 and len(pending) > 1:
                        emit_stage2(pending.pop(0))

                # PSUM -> SBUF pure cast, one ACT instruction per row
                lsb = lsb_pool.tile([128, 2, S], HP, tag="lsb")
                nc.scalar.activation(lsb[:], psl[:], copyf)

                # stage 2 is emitted one row late (software pipeline) so the
                # PE never waits on the ACT/DVE work of the same row.
                pending.append((r, lsb, mg))
            while pending:
                emit_stage2(pending.pop(0))

    if split_waits:
        _split_sync_waits(nc)
    return nc


def _host_prep(hidden, W, b, seg):
    """Layout/encoding prep: fp8 e3m4 per-token quantization of hidden,
    1/count of the integer segment ids, partition-major packing."""
    h = np.ascontiguousarray(hidden[:, 1:, :], dtype=np.float32)
    absmax = np.abs(h).max(axis=2, keepdims=True)  # [B, T, 1]
    s_t = (14.0 / np.maximum(absmax, 1e-30)).astype(np.float32)
    h8 = (h * s_t).astype(ml_dtypes.float8_e3m4)
    # [core][p, r, k, t] with p the SBUF partition (= h % 128 within chunk k)
    h8 = h8.reshape(N_CORES, RPC, T, KCH, 128)
    hiddenT = np.ascontiguousarray(h8.transpose(0, 4, 1, 3, 2))

    seg = np.asarray(seg)
    counts = np.zeros((B, T), dtype=np.int64)
    rows = np.arange(B)[:, None]
    np.add.at(counts, (rows, seg), 1)
    g = (1.0 / np.maximum(counts, 1))[rows, seg].astype(np.float32)  # [B, T]
    gs = (g / s_t[:, :, 0]).astype(np.float32)
    segf = seg.astype(np.float32)

    # partition-major packing: [core][p, c, r] = value at (row0+r, 128c+p)
    def pack(x):
        x4 = x.reshape(N_CORES, RPC, 2, 128)  # [core, r, c, p]
        return np.ascontiguousarray(x4.transpose(0, 3, 2, 1))

    segt = pack(segf)
    gst = pack(gs)
    w16 = np.asarray(W, dtype=np.float32).astype(np.float16).reshape(KCH, 128, S)
    w_in = np.ascontiguousarray(w16.transpose(1, 0, 2))  # [128, KCH, S]
    b_in = np.ascontiguousarray(b, dtype=np.float32).astype(np.float16).reshape(1, S)
    return hiddenT, w_in, b_in, segt, gst


_CACHE = {}


def kernel(hidden, W, b, seg):
    hiddenT, w_in, b_in, segt, gst = _host_prep(hidden, W, b, seg)
    with_bias = bool(np.any(b_in != 0.0))

    key = ("prog", with_bias)
    if key not in _CACHE:
        _CACHE[key] = _build_program(with_bias=with_bias)
    nc = _CACHE[key]

    in_maps = []
    for c in range(N_CORES):
        in_maps.append(
            {
                "hiddent": hiddenT[c],
                "w": w_in,
                "bvec": b_in,
                "segt": segt[c],
                "gst": gst[c],
            }
        )
    res = run_bass_kernel_spmd(nc, in_maps, core_ids=list(range(N_CORES)))
    # device layout: out1 [128, RPC, T] (s 0:128) + out2 [2, RPC, T] (s 128:130)
    out = np.empty((B, S, T), dtype=np.float32)
    for c in range(N_CORES):
        o1 = np.asarray(res.results[c]["out1"], dtype=np.float32)
        o2 = np.asarray(res.results[c]["out2"], dtype=np.float32)
        out[c * RPC : (c + 1) * RPC, 0:128, :] = o1.transpose(1, 0, 2)
        out[c * RPC : (c + 1) * RPC, 128:S, :] = o2.transpose(1, 0, 2)
    return out


# revision 28
# speedup vs baseline: 1.0165x; 1.0165x over previous
"""Trainium2 Bass kernel for nn_ModelIAS_53618371724066 (segment_reduce).

Computes, for each batch row b:
    logits = hidden[b, 1:, :] @ W + b_vec          # [T, S]
    merged[w, :] = mean over {t : seg[b,t] == w} of logits[t, :]   (0 if empty)
    out[b] = merged.T                               # [S, T]

Strategy (data-parallel over batch, 32 rows per core on 8 cores):
  - hidden is quantized host-side to fp8 e3m4 with a per-token scale s_t
    (absmax -> 14.0) so input DMA traffic halves vs fp16; W stays fp16
    (its small magnitudes fall into the e3m4 denormal range).  The matmul
    runs mixed fp8 x fp16 at the bf16 stream rate with fp8 FWL weight
    loads.  Measured end-to-end max rel err ~1.4e-2 vs the 2e-2 gate.
  - The scatter matrix Mg[t, w] = (seg[t] == w) * g[t]/s_t is built in
    ONE DVE tensor_scalar per t-chunk using the two-op form
    (is_equal then mult) -- folding the mean weight g and the fp8
    de-scale into Mg.  This makes the PSUM->SBUF logits copy a pure
    cast, done as a single ACT instruction per row over [128, 2, S].
  - Stage 1 (PE): logits[t_chunk, s] = sum_k hidT[k-chunk].T @ W[k-chunk]
    accumulated in one fp32 PSUM bank per row ([128, 2, S]).
  - Stage 2 (PE): out[s, w] = sum_c lsb_c.T @ Mg_c.  s-channels 0:128 go
    to po1 [128, T]; the 130-wide remainder (2 channels) accumulates into
    a shared [2, 2, T] bank per row-pair so its PSUM->SBUF copy and DMA
    amortize over 2 rows.
  - Outputs leave the chip in fp16 (host converts to f32): po1 is cast by
    DVE into a [128, 4, T] staging tile (one DMA per 4 rows), po2 by ACT.
  - A ~4us burst of dummy matmuls at program start warms the PE HAM clock
    gate (1.2 -> 2.4 GHz) before real work arrives.
  - All hidden DMAs are issued up front on the GPSIMD ring (whole input
    fits SBUF); outputs go on the SP ring; constants on the ACT ring.
  - Per-instruction sem-waits are legalized for the pinned walrus by
    _split_sync_waits.
"""

import numpy as np
import ml_dtypes

import concourse.bass as bass
import concourse.tile as tile
from concourse import mybir
from concourse.bass_utils import run_bass_kernel_spmd

B, T, H, S = 256, 256, 768, 130
N_CORES = 8
RPC = B // N_CORES  # rows per core
KCH = H // 128  # k chunks of the hidden dim
F32 = mybir.dt.float32
HP = mybir.dt.float16
FP8 = mybir.dt.float8e3  # e3m4
N_WARM = 8
ROWS_PER_HDMA = 4
HDMA_GROUPS = [2, 2, 4, 4, 4, 4, 4, 4, 4]


def _split_sync_waits(nc):
    """The pinned walrus build rejects instructions carrying more than one
    sync-wait command ("Too many sync wait commands", setupSyncWait).  Keep
    one wait per instruction and hoist the rest onto NoOps inserted just
    before it on the same engine (same semantics: all waits still execute
    before the instruction, in stream order)."""
    for f in nc.m.functions:
        for blk in f.blocks:
            il = blk.instructions
            i = 0
            while i < len(il):
                inst = il[i]
                si = inst.sync_info
                if si is not None and si.on_wait and len(si.on_wait) >= 2:
                    waits = list(si.on_wait)
                    keep = [waits.pop()]
                    pos = i
                    for j, w in enumerate(waits):
                        nop = mybir.InstNoOp(name=f"{inst.name}_ws{j}", ins=[], outs=[])
                        nop.engine = inst.engine
                        nop.sync_info = mybir.SyncInfo(on_wait=[w], on_update=[])
                        il.insert(pos, nop)
                        pos += 1
                        i += 1
                    inst.sync_info = mybir.SyncInfo(
                        on_wait=keep, on_update=list(si.on_update)
                    )
                i += 1


def _build_program(rpc=RPC, with_bias=False, split_waits=True):
    nc = bass.Bass("TRN2", target_bir_lowering=False, debug=False)

    hid = nc.dram_tensor("hiddent", [128, rpc, KCH, T], FP8, kind="ExternalInput")
    w_d = nc.dram_tensor("w", [128, KCH, S], HP, kind="ExternalInput")
    b_d = nc.dram_tensor("bvec", [1, S], HP, kind="ExternalInput")
    seg_d = nc.dram_tensor("segt", [128, 2, rpc], F32, kind="ExternalInput")
    g_d = nc.dram_tensor("gst", [128, 2, rpc], F32, kind="ExternalInput")
    out1_d = nc.dram_tensor("out1", [128, rpc, T], HP, kind="ExternalOutput")
    out2_d = nc.dram_tensor("out2", [2, rpc, T], HP, kind="ExternalOutput")

    eq = mybir.AluOpType.is_equal
    mul = mybir.AluOpType.mult
    copyf = mybir.ActivationFunctionType.Copy
    assert rpc % ROWS_PER_HDMA == 0
    ngrp = rpc // ROWS_PER_HDMA
    with tile.TileContext(nc) as tc:
        with (
            tc.tile_pool(name="const", bufs=1) as const_pool,
            tc.tile_pool(name="hid", bufs=len(HDMA_GROUPS)) as hid_pool,
            tc.tile_pool(name="mgp", bufs=3) as mg_pool,
            tc.tile_pool(name="lsbp", bufs=3) as lsb_pool,
            tc.tile_pool(name="ob1p", bufs=3) as ob1_pool,
            tc.tile_pool(name="ob2p", bufs=4) as ob2_pool,
            tc.tile_pool(name="warm", bufs=1, space=bass.MemorySpace.PSUM) as warm_pool,
            tc.tile_pool(name="psl", bufs=2, space=bass.MemorySpace.PSUM) as psl_pool,
            tc.tile_pool(name="po1", bufs=2, space=bass.MemorySpace.PSUM) as po1_pool,
            tc.tile_pool(name="po2", bufs=2, space=bass.MemorySpace.PSUM) as po2_pool,
        ):
            # --- HAM warmup: keep the PE busy from t~0 so the clock gate
            # opens (K=8/8) before real matmuls arrive.  Dummy data.
            wz = const_pool.tile([128, 512], HP)
            nc.vector.memset(wz[:], 0.0)
            pwu = warm_pool.tile([64, 512], F32)
            for i in range(N_WARM):
                nc.tensor.matmul(
                    pwu[:, :],
                    wz[:, 0:64],
                    wz[:, :],
                    start=True,
                    stop=True,
                    skip_group_check=True,
                )

            # --- all input DMAs ride ONE SP-ring queue in priority order:
            # tiny constants first (so per-row deps clear by ~9us), then the
            # whole hidden tensor.  The first two hidden chunks are 2 rows
            # each so row 0 can start as early as possible.  The SP engine
            # has no per-row duties, so trigger instructions blocking on
            # DMA-ring capacity cost nothing.
            hts = []  # (row_start, n_rows, tile)
            ht0 = hid_pool.tile([128, HDMA_GROUPS[0], KCH, T], FP8, tag="ht0", name="htt")
            nc.sync.dma_start(ht0[:], hid.ap()[:, 0 : HDMA_GROUPS[0]])
            hts.append((0, HDMA_GROUPS[0], ht0))
            wt = const_pool.tile([128, KCH, S], HP)
            nc.sync.dma_start(wt[:], w_d.ap()[:])
            # tiny per-row constants ride the otherwise-idle GPSIMD ring so
            # they don't queue behind hidden chunks
            segt = const_pool.tile([128, 2, rpc], F32)
            nc.gpsimd.dma_start(segt[:], seg_d.ap()[:])
            gst = const_pool.tile([128, 2, rpc], F32)
            nc.gpsimd.dma_start(gst[:], g_d.ap()[:])
            row0 = HDMA_GROUPS[0]
            for nrow in HDMA_GROUPS[1:]:
                ht = hid_pool.tile([128, nrow, KCH, T], FP8, tag=f"ht{nrow}", name="htt")
                nc.sync.dma_start(ht[:], hid.ap()[:, row0 : row0 + nrow])
                hts.append((row0, nrow, ht))
                row0 += nrow
            assert row0 == rpc

            def hid_slice(r):
                for row0, nrow, ht in hts:
                    if row0 <= r < row0 + nrow:
                        return ht[:, r - row0]
                raise AssertionError

            # dummy ACTIVATE pulls the lazy ACT table load into the idle
            # startup window instead of the first row's critical path
            wz2 = const_pool.tile([1, 8], HP)
            nc.scalar.copy(wz2[:], wz[0:1, 0:8])
            iota_i = const_pool.tile([128, T], mybir.dt.int32)
            nc.gpsimd.iota(iota_i[:], pattern=[[1, T]], base=0, channel_multiplier=0)
            iota16 = const_pool.tile([128, T], HP)
            nc.vector.tensor_copy(iota16[:], iota_i[:])
            if with_bias:
                ones = const_pool.tile([1, 128], HP)
                nc.vector.memset(ones[:], 1.0)
                bsb = const_pool.tile([1, S], HP)
                nc.scalar.dma_start(bsb[:], b_d.ap()[:])

            pending = []
            po2s = {}
            ob1s = {}
            ob2s = {}

            def emit_stage2(item):
                pr, plsb, pmg = item
                pair, rr2 = divmod(pr, 2)
                g4, rr4 = divmod(pr, 4)
                po1 = po1_pool.tile([128, T], F32, tag="po1")
                if rr2 == 0:
                    po2s[pair] = po2_pool.tile([128, 2, T], F32, tag="po2", name="po2t")
                po2 = po2s[pair]
                # po2 uses a full-width stationary (s-channels 2:130) so all
                # stage-2 matmuls are unmasked M=128 and pipeline back-to-back;
                # only its last 2 partitions (s=128,129) are kept.
                for c in range(2):
                    nc.tensor.matmul(
                        po1[:],
                        plsb[:, c, 0:128],
                        pmg[:, c, :],
                        start=(c == 0),
                        stop=(c == 1),
                        skip_group_check=True,
                    )
                for c in range(2):
                    nc.tensor.matmul(
                        po2[:, rr2, :],
                        plsb[:, c, 2:S],
                        pmg[:, c, :],
                        start=(c == 0),
                        stop=(c == 1),
                        skip_group_check=True,
                    )
                g8, rr8 = divmod(pr, 8)
                if rr8 == 0:
                    ob1s[g8] = ob1_pool.tile([128, 8, T], HP, tag="ob1", name="ob1t")
                ob1 = ob1s[g8]
                nc.vector.tensor_copy(ob1[:, rr8, :], po1[:])
                if g8 == 3 and rr8 == 3:
                    # split the last group's DMAs so the final transfer is small
                    nc.sync.dma_start(
                        out1_d.ap()[:, 24:28], ob1[:, 0:4, :]
                    )
                if g8 == 3 and rr8 == 5:
                    nc.sync.dma_start(
                        out1_d.ap()[:, 28:30], ob1[:, 4:6, :]
                    )
                if rr8 == 7:
                    if g8 == 3:
                        nc.sync.dma_start(
                            out1_d.ap()[:, 30:32], ob1[:, 6:8, :]
                        )
                    else:
                        nc.sync.dma_start(
                            out1_d.ap()[:, 8 * g8 : 8 * (g8 + 1)], ob1[:]
                        )
                    del ob1s[g8]
                if rr2 == 1:
                    if rr4 == 1:
                        ob2s[g4] = ob2_pool.tile([128, 4, T], HP, tag="ob2", name="ob2t")
                    ob2 = ob2s[g4]
                    nc.scalar.copy(ob2[:, 2 * (pair % 2) : 2 * (pair % 2) + 2, :], po2[:])
                    del po2s[pair]
                    if rr4 == 3:
                        nc.sync.dma_start(
                            out2_d.ap()[:, 4 * g4 : 4 * (g4 + 1)],
                            ob2[126:128, :, :],
                        )
                        del ob2s[g4]

            for r in range(rpc):
                ht_r = hid_slice(r)

                # Mg[t, w] = (seg[t] == w) * g[t]/s_t in fp16, one DVE
                # two-op tensor_scalar per t-chunk
                mg = mg_pool.tile([128, 2, T], HP, tag="mg")
                for c in range(2):
                    nc.vector.tensor_scalar(
                        mg[:, c, :],
                        iota16[:],
                        segt[:, c, r : r + 1],
                        gst[:, c, r : r + 1],
                        eq,
                        mul,
                    )

                # stage 1: scaled logits for both t-chunks into one PSUM bank
                psl = psl_pool.tile([128, 2, S], F32, tag="psl")
                for c in range(2):
                    for k in range(KCH):
                        nc.tensor.matmul(
                            psl[:, c, :],
                            ht_r[:, k, 128 * c : 128 * (c + 1)],
                            wt[:, k, :],
                            start=(k == 0),
                            stop=(k == KCH - 1 and not with_bias),
                            skip_group_check=True,
                        )
                    if with_bias:
                        nc.tensor.matmul(
                            psl[:, c, :], ones[:], bsb[:], start=False, stop=True,
                            skip_group_check=True,
                        )

                # PSUM -> SBUF pure cast, one ACT instruction per row
                lsb = lsb_pool.tile([128, 2, S], HP, tag="lsb")
                nc.scalar.activation(lsb[:], psl[:], copyf)

                # stage 2 is emitted one row late (software pipeline) so the
                # PE never waits on the ACT/DVE work of the same row.
                pending.append((r, lsb, mg))
                if len(pending) > 1:
                    emit_stage2(pending.pop(0))
            while pending:
                emit_stage2(pending.pop(0))

    if split_waits:
        _split_sync_waits(nc)
    return nc


def _host_prep(hidden, W, b, seg):
    """Layout/encoding prep: fp8 e3m4 per-token quantization of hidden,
    1/count of the integer segment ids, partition-major packing."""
    h = np.ascontiguousarray(hidden[:, 1:, :], dtype=np.float32)
    absmax = np.abs(h).max(axis=2, keepdims=True)  # [B, T, 1]
    s_t = (14.0 / np.maximum(absmax, 1e-30)).astype(np.float32)
    h8 = (h * s_t).astype(ml_dtypes.float8_e3m4)
    # [core][p, r, k, t] with p the SBUF partition (= h % 128 within chunk k)
    h8 = h8.reshape(N_CORES, RPC, T, KCH, 128)
    hiddenT = np.ascontiguousarray(h8.transpose(0, 4, 1, 3, 2))

    seg = np.asarray(seg)
    counts = np.zeros((B, T), dtype=np.int64)
    rows = np.arange(B)[:, None]
    np.add.at(counts, (rows, seg), 1)
    g = (1.0 / np.maximum(counts, 1))[rows, seg].astype(np.float32)  # [B, T]
    gs = (g / s_t[:, :, 0]).astype(np.float32)
    segf = seg.astype(np.float32)

    # partition-major packing: [core][p, c, r] = value at (row0+r, 128c+p)
    def pack(x):
        x4 = x.reshape(N_CORES, RPC, 2, 128)  # [core, r, c, p]
        return np.ascontiguousarray(x4.transpose(0, 3, 2, 1))

    segt = pack(segf)
    gst = pack(gs)
    w16 = np.asarray(W, dtype=np.float32).astype(np.float16).reshape(KCH, 128, S)
    w_in = np.ascontiguousarray(w16.transpose(1, 0, 2))  # [128, KCH, S]
    b_in = np.ascontiguousarray(b, dtype=np.float32).astype(np.float16).reshape(1, S)
    return hiddenT, w_in, b_in, segt, gst


_CACHE = {}


def kernel(hidden, W, b, seg):
    hiddenT, w_in, b_in, segt, gst = _host_prep(hidden, W, b, seg)
    with_bias = bool(np.any(b_in != 0.0))

    key = ("prog", with_bias)
    if key not in _CACHE:
        _CACHE[key] = _build_program(with_bias=with_bias)
    nc = _CACHE[key]

    in_maps = []
    for c in range(N_CORES):
        in_maps.append(
            {
                "hiddent": hiddenT[c],
                "w": w_in,
                "bvec": b_in,
                "segt": segt[c],
                "gst": gst[c],
            }
        )
    res = run_bass_kernel_spmd(nc, in_maps, core_ids=list(range(N_CORES)))
    # device layout: out1 [128, RPC, T] (s 0:128) + out2 [2, RPC, T] (s 128:130)
    out = np.empty((B, S, T), dtype=np.float32)
    for c in range(N_CORES):
        o1 = np.asarray(res.results[c]["out1"], dtype=np.float32)
        o2 = np.asarray(res.results[c]["out2"], dtype=np.float32)
        out[c * RPC : (c + 1) * RPC, 0:128, :] = o1.transpose(1, 0, 2)
        out[c * RPC : (c + 1) * RPC, 128:S, :] = o2.transpose(1, 0, 2)
    return out
